# revision 23
# baseline (speedup 1.0000x reference)
# Multi-head attention (B=2, S=2048, D=1024, H=16, dh=64) on 8 TRN2 NeuronCores.
# Sharding: core = batch * 4 + head_group; each core handles one batch and 4 heads.
# Host prep: q/k/v transposed to feature-major bf16; weight slices transposed bf16.
# Kernel: projections -> scoresT = K^T@Q (2-head row-packed, K=64, shared 2-bank
# psum tile) -> one exp per head-pair (ScalarE, scale fused) -> multiplicative
# keep-mask on partial tiles only (GpSimd) -> PV with ones-augmented V (M=65)
# giving softmax denominators for free -> unnormalized att evacuated to SBUF,
# normalization deferred: strided-partition reciprocal + K=1 broadcast matmuls +
# in-place GpSimd multiply -> row-parallel Wo; partial outputs summed on host.
import numpy as np
import ml_dtypes

import concourse.bass as bass
import concourse.tile as tile
from concourse import bacc, mybir
from concourse import bass_utils

B, S, D = 2, 2048, 1024
H, DH = 16, 64
NCORES = 8
GROUPS = 4            # head groups per batch (cores per batch)
HPG = 4               # heads per group
FPG = HPG * DH        # 256 features per group
SQ_T, SK_T = 512, 128
NSQ, NSK = S // SQ_T, S // SK_T
NCH = D // 128        # 8 contraction chunks of d_model
BF16 = ml_dtypes.bfloat16

_BUILT = {}


def _classify(mask):
    """Per-tile mask classification in scoresT space: tile (i, j) covers
    k in [i*128, (i+1)*128), q in [j*512, (j+1)*512)."""
    keep_t = (~np.asarray(mask, dtype=bool)).T  # [k, q], True = attend
    cls = {}
    ptiles = []
    for j in range(NSQ):
        for i in range(NSK):
            sub = keep_t[i * SK_T:(i + 1) * SK_T, j * SQ_T:(j + 1) * SQ_T]
            if not sub.any():
                cls[(i, j)] = "skip"
            elif sub.all():
                cls[(i, j)] = ("full", 0, SQ_T)
            else:
                # column bounding range with any unmasked entry
                cols = np.flatnonzero(sub.any(axis=0))
                cls[(i, j)] = (len(ptiles), int(cols[0]), int(cols[-1]) + 1)
                ptiles.append(np.ascontiguousarray(sub.astype(BF16)))
    return cls, ptiles


def _build(cls, n_ptiles):
    nc = bacc.Bacc("TRN2", target_bir_lowering=False, debug=False)
    dt = mybir.dt
    f32, bf = dt.float32, dt.bfloat16
    EXP = mybir.ActivationFunctionType.Exp

    xq = nc.dram_tensor("xqt", [D, S], bf, kind="ExternalInput").ap()
    xk = nc.dram_tensor("xkt", [D, S], bf, kind="ExternalInput").ap()
    xv = nc.dram_tensor("xvt", [D, S], bf, kind="ExternalInput").ap()
    wq = nc.dram_tensor("wqt", [D, FPG], bf, kind="ExternalInput").ap()
    wk = nc.dram_tensor("wkt", [D, FPG], bf, kind="ExternalInput").ap()
    wv = nc.dram_tensor("wvt", [D, FPG], bf, kind="ExternalInput").ap()
    wo = nc.dram_tensor("wot", [FPG, D], bf, kind="ExternalInput").ap()
    kp = nc.dram_tensor("keep", [max(n_ptiles, 1) * SK_T, SQ_T], bf,
                        kind="ExternalInput").ap()
    out = nc.dram_tensor("out", [S, D], f32, kind="ExternalOutput").ap()

    xq_v = xq.rearrange("(c p) s -> p c s", p=128)
    xk_v = xk.rearrange("(c p) s -> p c s", p=128)
    xv_v = xv.rearrange("(c p) s -> p c s", p=128)
    wq_v = wq.rearrange("(c p) f -> p c f", p=128)
    wk_v = wk.rearrange("(c p) f -> p c f", p=128)
    wv_v = wv.rearrange("(c p) f -> p c f", p=128)
    wo_v = wo.rearrange("(c p) o -> p c o", p=128)
    kp_v = kp.rearrange("(n p) s -> p n s", p=128)
    out_v = out.rearrange("(r p) o -> r p o", p=128)

    with tile.TileContext(nc) as tc:
        with (
            tc.tile_pool(name="consts", bufs=1) as consts,
            tc.tile_pool(name="x", bufs=2) as xpool,
            tc.tile_pool(name="sc", bufs=2, space="PSUM") as sc_ps,
            tc.tile_pool(name="pv", bufs=2, space="PSUM") as pv_ps,
            tc.tile_pool(name="aux", bufs=2, space="PSUM") as aux_ps,
            tc.tile_pool(name="work", bufs=4) as work,
            tc.tile_pool(name="probs", bufs=6) as prpool,
            tc.tile_pool(name="small", bufs=2) as small,
        ):
            # critical-path DMAs first (sync queue): wq/wk/wv then x tiles
            wq_sb = consts.tile([128, NCH, FPG], bf)
            nc.sync.dma_start(wq_sb[:], wq_v[:])
            wk_sb = consts.tile([128, NCH, FPG], bf)
            nc.sync.dma_start(wk_sb[:], wk_v[:])
            wv_sb = consts.tile([128, NCH, FPG], bf)
            nc.scalar.dma_start(wv_sb[:], wv_v[:])
            wo_sb = consts.tile([128, FPG // 128, D], bf)
            keep_sb = consts.tile([128, max(n_ptiles, 1), SQ_T], bf)
            ones_sb = consts.tile([128, 128], bf)
            nc.vector.memset(ones_sb[:], 1.0)
            qh_sb = consts.tile([128, 2, S], bf)
            kh_sb = consts.tile([128, 2, S], bf)
            vh_sb = consts.tile([128, NSK, HPG, DH + 1], bf)
            nc.vector.memset(vh_sb[:], 1.0)  # ones column (col DH) survives
            att_sb = consts.tile([128, 2, S], bf)
            l4 = consts.tile([128, SQ_T], f32)
            r4 = consts.tile([128, SQ_T], bf)
            nc.vector.memset(l4[:], 1.0)  # unused partitions -> r = 1

            # ---- Phase A: projections (feature-major qh/kh, token-major vh) ----
            for t in range(NSQ):
                sl = bass.ts(t, SQ_T)
                xq_t = xpool.tile([128, NCH, SQ_T], bf, tag="xq")
                nc.sync.dma_start(xq_t[:], xq_v[:, :, sl])
                xk_t = xpool.tile([128, NCH, SQ_T], bf, tag="xk")
                nc.scalar.dma_start(xk_t[:], xk_v[:, :, sl])
                xv_t = xpool.tile([128, NCH, SQ_T], bf, tag="xv")
                nc.scalar.dma_start(xv_t[:], xv_v[:, :, sl])
                if t == 1:
                    nc.scalar.dma_start(keep_sb[:], kp_v[:])
                    nc.sync.dma_start(wo_sb[:], wo_v[:])
                # q/k projection chains, one aux bank per head-pair chain
                for wsb, hout in ((wq_sb, qh_sb), (wk_sb, kh_sb)):
                    for hp in range(2):
                        hsl = bass.ts(hp, 128)
                        ps = aux_ps.tile([128, SQ_T], f32, tag="aux")
                        for c in range(NCH):
                            nc.tensor.matmul(ps[:], wsb[:, c, hsl],
                                             xq_t[:, c, :] if wsb is wq_sb
                                             else xk_t[:, c, :],
                                             start=(c == 0),
                                             stop=(c == NCH - 1))
                        nc.vector.tensor_copy(hout[:, hp, sl], ps[:])
                for s4 in range(SQ_T // SK_T):
                    i = t * (SQ_T // SK_T) + s4
                    psv = aux_ps.tile([128, SQ_T], f32, tag="aux")
                    for c in range(NCH):
                        nc.tensor.matmul(psv[:, 0:FPG],
                                         xv_t[:, c, bass.ts(s4, SK_T)],
                                         wv_sb[:, c, :],
                                         start=(c == 0), stop=(c == NCH - 1))
                    nc.vector.tensor_copy(
                        vh_sb[:, i, :, 0:DH],
                        psv[:, 0:FPG].rearrange("p (h d) -> p h d", h=HPG))

            # ---- Phase B: attention per q-tile ----
            for j in range(NSQ):
                jsl = bass.ts(j, SQ_T)
                kept = [i for i in range(NSK) if cls[(i, j)] != "skip"]
                for hp in range(2):
                    pv0 = pv_ps.tile([DH + 1, SQ_T], f32, tag="pv")
                    pv1 = pv_ps.tile([DH + 1, SQ_T], f32, tag="pv")
                    for n, i in enumerate(kept):
                        isl = bass.ts(i, SK_T)
                        c, c0, c1 = cls[(i, j)]
                        if n == 0:
                            c0, c1 = 0, SQ_T  # first tile must cover the bank
                        qsl = bass.ds(j * SQ_T + c0, c1 - c0)
                        sc = sc_ps.tile([128, 2, SQ_T], f32, tag="sc")
                        nc.tensor.matmul(sc[:, 0, c0:c1], kh_sb[0:64, hp, isl],
                                         qh_sb[0:64, hp, qsl], start=True,
                                         stop=True, tile_position=(0, 0))
                        nc.tensor.matmul(sc[:, 1, c0:c1], kh_sb[64:128, hp, isl],
                                         qh_sb[64:128, hp, qsl], start=True,
                                         stop=True, tile_position=(64, 0))
                        pr = prpool.tile([128, 2, SQ_T], bf, tag="probs")
                        nc.scalar.activation(pr[:, :, c0:c1], sc[:, :, c0:c1],
                                             EXP, scale=0.125)
                        if c != "full":
                            nc.vector.tensor_mul(pr[:, 0, c0:c1],
                                                 pr[:, 0, c0:c1],
                                                 keep_sb[:, c, c0:c1])
                            nc.vector.tensor_mul(pr[:, 1, c0:c1],
                                                 pr[:, 1, c0:c1],
                                                 keep_sb[:, c, c0:c1])
                        nc.tensor.matmul(pv0[:, c0:c1],
                                         vh_sb[:, i, 2 * hp + 0, :],
                                         pr[:, 0, c0:c1], start=(n == 0),
                                         stop=(n == len(kept) - 1))
                        nc.tensor.matmul(pv1[:, c0:c1],
                                         vh_sb[:, i, 2 * hp + 1, :],
                                         pr[:, 1, c0:c1], start=(n == 0),
                                         stop=(n == len(kept) - 1))
                    # evacuate psum fast: unnormalized att + denominators
                    nc.vector.tensor_copy(att_sb[0:64, hp, jsl], pv0[0:64, :])
                    nc.vector.tensor_copy(att_sb[64:128, hp, jsl], pv1[0:64, :])
                    p0 = 64 * hp
                    nc.vector.tensor_copy(l4[p0:p0 + 1, :], pv0[DH:DH + 1, :])
                    nc.vector.tensor_copy(l4[p0 + 32:p0 + 33, :],
                                          pv1[DH:DH + 1, :])
                # deferred normalization: r = 1/l with head denominators at
                # partitions {0,32,64,96}; full-partition op (cost = free size)
                with nc.allow_low_precision(reason="bf16 softmax denom"):
                    nc.vector.reciprocal(r4[:], l4[:])
                rb_sb = work.tile([128, 2, SQ_T], bf, tag="rbsb")
                for hp in range(2):
                    rb = aux_ps.tile([128, SQ_T], f32, tag="aux")
                    pe, po_ = 64 * hp, 64 * hp + 32
                    nc.tensor.matmul(rb[0:64, :], ones_sb[pe:pe + 1, 0:64],
                                     r4[pe:pe + 1, :], start=True, stop=True,
                                     tile_position=(pe, 0))
                    nc.tensor.matmul(rb[64:128, :], ones_sb[po_:po_ + 1, 64:128],
                                     r4[po_:po_ + 1, :], start=True, stop=True,
                                     tile_position=(po_, 64))
                    nc.vector.tensor_copy(rb_sb[:, hp, :], rb[:])
                    nc.vector.tensor_mul(att_sb[:, hp, jsl], att_sb[:, hp, jsl],
                                         rb_sb[:, hp, :])
                # ---- output projection for this q-tile ----
                for t4 in range(SQ_T // 128):
                    r_ = j * (SQ_T // 128) + t4
                    tsl = bass.ds(j * SQ_T + t4 * 128, 128)
                    for o in range(2):
                        po = aux_ps.tile([128, SQ_T], f32, tag="aux")
                        for hp in range(2):
                            nc.tensor.matmul(po[:], att_sb[:, hp, tsl],
                                             wo_sb[:, hp, bass.ts(o, 512)],
                                             start=(hp == 0), stop=(hp == 1))
                        ost = work.tile([128, 512], f32, tag="ost")
                        nc.vector.tensor_copy(ost[:], po[:])
                        nc.sync.dma_start(out_v[r_, :, bass.ts(o, 512)], ost[:])

    nc.compile()
    return nc


def _get_nc(mask):
    key = hash(np.asarray(mask, dtype=bool).tobytes())
    if key not in _BUILT:
        cls, ptiles = _classify(mask)
        _BUILT[key] = (_build(cls, len(ptiles)), cls, ptiles)
    return _BUILT[key]


def _kernel_impl(q, k, v, attn_mask, Wq, Wk, Wv, Wo, trace=False):
    q = np.asarray(q, dtype=np.float32)
    k = np.asarray(k, dtype=np.float32)
    v = np.asarray(v, dtype=np.float32)
    nc, cls, ptiles = _get_nc(attn_mask)

    if ptiles:
        keep_packed = np.concatenate(ptiles, axis=0)
    else:
        keep_packed = np.zeros((SK_T, SQ_T), dtype=BF16)

    xt = {}
    for b in range(B):
        xt[("q", b)] = np.ascontiguousarray(q[b].T.astype(BF16))
        xt[("k", b)] = np.ascontiguousarray(k[b].T.astype(BF16))
        xt[("v", b)] = np.ascontiguousarray(v[b].T.astype(BF16))
    wslices = {}
    for g in range(GROUPS):
        fsl = slice(g * FPG, (g + 1) * FPG)
        wslices[("wq", g)] = np.ascontiguousarray(Wq[fsl, :].T.astype(BF16))
        wslices[("wk", g)] = np.ascontiguousarray(Wk[fsl, :].T.astype(BF16))
        wslices[("wv", g)] = np.ascontiguousarray(Wv[fsl, :].T.astype(BF16))
        wslices[("wo", g)] = np.ascontiguousarray(Wo[:, fsl].T.astype(BF16))

    in_maps = []
    for core in range(NCORES):
        b, g = core // GROUPS, core % GROUPS
        in_maps.append({
            "xqt": xt[("q", b)], "xkt": xt[("k", b)], "xvt": xt[("v", b)],
            "wqt": wslices[("wq", g)], "wkt": wslices[("wk", g)],
            "wvt": wslices[("wv", g)], "wot": wslices[("wo", g)],
            "keep": keep_packed,
        })

    res = bass_utils.run_bass_kernel_spmd(
        nc, in_maps, core_ids=list(range(NCORES)), trace=trace)

    out = np.zeros((B, S, D), dtype=np.float32)
    for core in range(NCORES):
        out[core // GROUPS] += res.results[core]["out"]
    return out, res


def kernel(q, k, v, attn_mask, Wq, Wk, Wv, Wo):
    out, _ = _kernel_impl(q, k, v, attn_mask, Wq, Wk, Wv, Wo)
    return out


# revision 24
# speedup vs baseline: 1.0487x; 1.0487x over previous
# Multi-head attention (B=2, S=2048, D=1024, H=16, dh=64) on 8 TRN2 NeuronCores.
# Sharding: core = batch * 4 + head_group; each core handles one batch and 4 heads.
# Host prep: q/k/v transposed to feature-major bf16; weight slices transposed bf16.
# Kernel: projections -> scoresT = K^T@Q (2-head row-packed, K=64, shared 2-bank
# psum tile) -> one exp per head-pair (ScalarE, scale fused) -> multiplicative
# keep-mask on partial tiles only (GpSimd) -> PV with ones-augmented V (M=65)
# giving softmax denominators for free -> unnormalized att evacuated to SBUF,
# normalization deferred: strided-partition reciprocal + K=1 broadcast matmuls +
# in-place GpSimd multiply -> row-parallel Wo; partial outputs summed on host.
import numpy as np
import ml_dtypes

import concourse.bass as bass
import concourse.tile as tile
from concourse import bacc, mybir
from concourse import bass_utils

B, S, D = 2, 2048, 1024
H, DH = 16, 64
NCORES = 8
GROUPS = 4            # head groups per batch (cores per batch)
HPG = 4               # heads per group
FPG = HPG * DH        # 256 features per group
SQ_T, SK_T = 512, 128
NSQ, NSK = S // SQ_T, S // SK_T
NCH = D // 128        # 8 contraction chunks of d_model
BF16 = ml_dtypes.bfloat16

_BUILT = {}


def _classify(mask):
    """Per-tile mask classification in scoresT space: tile (i, j) covers
    k in [i*128, (i+1)*128), q in [j*512, (j+1)*512)."""
    keep_t = (~np.asarray(mask, dtype=bool)).T  # [k, q], True = attend
    cls = {}
    ptiles = []
    for j in range(NSQ):
        for i in range(NSK):
            sub = keep_t[i * SK_T:(i + 1) * SK_T, j * SQ_T:(j + 1) * SQ_T]
            if not sub.any():
                cls[(i, j)] = "skip"
            elif sub.all():
                cls[(i, j)] = ("full", 0, SQ_T)
            else:
                # column bounding range with any unmasked entry
                cols = np.flatnonzero(sub.any(axis=0))
                cls[(i, j)] = (len(ptiles), int(cols[0]), int(cols[-1]) + 1)
                ptiles.append(np.ascontiguousarray(sub.astype(BF16)))
    return cls, ptiles


def _build(cls, n_ptiles):
    nc = bacc.Bacc("TRN2", target_bir_lowering=False, debug=False)
    dt = mybir.dt
    f32, bf = dt.float32, dt.bfloat16
    EXP = mybir.ActivationFunctionType.Exp

    xq = nc.dram_tensor("xqt", [D, S], bf, kind="ExternalInput").ap()
    xk = nc.dram_tensor("xkt", [D, S], bf, kind="ExternalInput").ap()
    xv = nc.dram_tensor("xvt", [D, S], bf, kind="ExternalInput").ap()
    wq = nc.dram_tensor("wqt", [D, FPG], bf, kind="ExternalInput").ap()
    wk = nc.dram_tensor("wkt", [D, FPG], bf, kind="ExternalInput").ap()
    wv = nc.dram_tensor("wvt", [D, FPG], bf, kind="ExternalInput").ap()
    wo = nc.dram_tensor("wot", [FPG, D], bf, kind="ExternalInput").ap()
    kp = nc.dram_tensor("keep", [max(n_ptiles, 1) * SK_T, SQ_T], bf,
                        kind="ExternalInput").ap()
    out = nc.dram_tensor("out", [S, D], f32, kind="ExternalOutput").ap()

    xq_v = xq.rearrange("(c p) s -> p c s", p=128)
    xk_v = xk.rearrange("(c p) s -> p c s", p=128)
    xv_v = xv.rearrange("(c p) s -> p c s", p=128)
    wq_v = wq.rearrange("(c p) f -> p c f", p=128)
    wk_v = wk.rearrange("(c p) f -> p c f", p=128)
    wv_v = wv.rearrange("(c p) f -> p c f", p=128)
    wo_v = wo.rearrange("(c p) o -> p c o", p=128)
    kp_v = kp.rearrange("(n p) s -> p n s", p=128)
    out_v = out.rearrange("(r p) o -> r p o", p=128)

    with tile.TileContext(nc) as tc:
        with (
            tc.tile_pool(name="consts", bufs=1) as consts,
            tc.tile_pool(name="x", bufs=2) as xpool,
            tc.tile_pool(name="sc", bufs=2, space="PSUM") as sc_ps,
            tc.tile_pool(name="pv", bufs=2, space="PSUM") as pv_ps,
            tc.tile_pool(name="aux", bufs=2, space="PSUM") as aux_ps,
            tc.tile_pool(name="work", bufs=4) as work,
            tc.tile_pool(name="probs", bufs=6) as prpool,
            tc.tile_pool(name="small", bufs=2) as small,
        ):
            # critical-path DMAs first (sync queue): wq/wk/wv then x tiles
            wq_sb = consts.tile([128, NCH, FPG], bf)
            nc.sync.dma_start(wq_sb[:], wq_v[:])
            wk_sb = consts.tile([128, NCH, FPG], bf)
            nc.sync.dma_start(wk_sb[:], wk_v[:])
            wv_sb = consts.tile([128, NCH, FPG], bf)
            nc.sync.dma_start(wv_sb[:], wv_v[:])
            wo_sb = consts.tile([128, FPG // 128, D], bf)
            nc.scalar.dma_start(wo_sb[:], wo_v[:])
            keep_sb = consts.tile([128, max(n_ptiles, 1), SQ_T], bf)
            nc.scalar.dma_start(keep_sb[:], kp_v[:])
            ones_sb = consts.tile([128, 128], bf)
            nc.vector.memset(ones_sb[:], 1.0)
            qh_sb = consts.tile([128, 2, S], bf)
            kh_sb = consts.tile([128, 2, S], bf)
            vh_sb = consts.tile([128, NSK, HPG, DH + 1], bf)
            nc.vector.memset(vh_sb[:], 1.0)  # ones column (col DH) survives
            att_sb = consts.tile([128, 2, S], bf)
            l4 = consts.tile([128, SQ_T], f32)
            r4 = consts.tile([128, SQ_T], bf)
            nc.vector.memset(l4[:], 1.0)  # unused partitions -> r = 1

            # ---- Phase A: projections (feature-major qh/kh, token-major vh) ----
            for t in range(NSQ):
                sl = bass.ts(t, SQ_T)
                xq_t = xpool.tile([128, NCH, SQ_T], bf, tag="xq")
                nc.sync.dma_start(xq_t[:], xq_v[:, :, sl])
                xk_t = xpool.tile([128, NCH, SQ_T], bf, tag="xk")
                nc.scalar.dma_start(xk_t[:], xk_v[:, :, sl])
                xv_t = xpool.tile([128, NCH, SQ_T], bf, tag="xv")
                nc.sync.dma_start(xv_t[:], xv_v[:, :, sl])
                # q/k projection chains, one aux bank per head-pair chain
                for wsb, hout in ((wq_sb, qh_sb), (wk_sb, kh_sb)):
                    for hp in range(2):
                        hsl = bass.ts(hp, 128)
                        ps = aux_ps.tile([128, SQ_T], f32, tag="aux")
                        for c in range(NCH):
                            nc.tensor.matmul(ps[:], wsb[:, c, hsl],
                                             xq_t[:, c, :] if wsb is wq_sb
                                             else xk_t[:, c, :],
                                             start=(c == 0),
                                             stop=(c == NCH - 1))
                        nc.vector.tensor_copy(hout[:, hp, sl], ps[:])
                for s4 in range(SQ_T // SK_T):
                    i = t * (SQ_T // SK_T) + s4
                    psv = aux_ps.tile([128, SQ_T], f32, tag="aux")
                    for c in range(NCH):
                        nc.tensor.matmul(psv[:, 0:FPG],
                                         xv_t[:, c, bass.ts(s4, SK_T)],
                                         wv_sb[:, c, :],
                                         start=(c == 0), stop=(c == NCH - 1))
                    nc.vector.tensor_copy(
                        vh_sb[:, i, :, 0:DH],
                        psv[:, 0:FPG].rearrange("p (h d) -> p h d", h=HPG))

            # ---- Phase B: attention per q-tile ----
            for j in range(NSQ):
                jsl = bass.ts(j, SQ_T)
                kept = [i for i in range(NSK) if cls[(i, j)] != "skip"]
                for hp in range(2):
                    pv0 = pv_ps.tile([DH + 1, SQ_T], f32, tag="pv")
                    pv1 = pv_ps.tile([DH + 1, SQ_T], f32, tag="pv")
                    for n, i in enumerate(kept):
                        isl = bass.ts(i, SK_T)
                        c, c0, c1 = cls[(i, j)]
                        if n == 0:
                            c0, c1 = 0, SQ_T  # first tile must cover the bank
                        qsl = bass.ds(j * SQ_T + c0, c1 - c0)
                        sc = sc_ps.tile([128, 2, SQ_T], f32, tag="sc")
                        nc.tensor.matmul(sc[:, 0, c0:c1], kh_sb[0:64, hp, isl],
                                         qh_sb[0:64, hp, qsl], start=True,
                                         stop=True, tile_position=(0, 0))
                        nc.tensor.matmul(sc[:, 1, c0:c1], kh_sb[64:128, hp, isl],
                                         qh_sb[64:128, hp, qsl], start=True,
                                         stop=True, tile_position=(64, 0))
                        pr = prpool.tile([128, 2, SQ_T], bf, tag="probs")
                        nc.scalar.activation(pr[:, :, c0:c1], sc[:, :, c0:c1],
                                             EXP, scale=0.125)
                        if c != "full":
                            nc.vector.tensor_mul(pr[:, 0, c0:c1],
                                                 pr[:, 0, c0:c1],
                                                 keep_sb[:, c, c0:c1])
                            nc.vector.tensor_mul(pr[:, 1, c0:c1],
                                                 pr[:, 1, c0:c1],
                                                 keep_sb[:, c, c0:c1])
                        nc.tensor.matmul(pv0[:, c0:c1],
                                         vh_sb[:, i, 2 * hp + 0, :],
                                         pr[:, 0, c0:c1], start=(n == 0),
                                         stop=(n == len(kept) - 1))
                        nc.tensor.matmul(pv1[:, c0:c1],
                                         vh_sb[:, i, 2 * hp + 1, :],
                                         pr[:, 1, c0:c1], start=(n == 0),
                                         stop=(n == len(kept) - 1))
                    # evacuate psum fast: unnormalized att + denominators
                    nc.vector.tensor_copy(att_sb[0:64, hp, jsl], pv0[0:64, :])
                    nc.vector.tensor_copy(att_sb[64:128, hp, jsl], pv1[0:64, :])
                    p0 = 64 * hp
                    nc.vector.tensor_copy(l4[p0:p0 + 1, :], pv0[DH:DH + 1, :])
                    nc.vector.tensor_copy(l4[p0 + 32:p0 + 33, :],
                                          pv1[DH:DH + 1, :])
                # deferred normalization: r = 1/l with head denominators at
                # partitions {0,32,64,96}; full-partition op (cost = free size)
                with nc.allow_low_precision(reason="bf16 softmax denom"):
                    nc.vector.reciprocal(r4[:], l4[:])
                rb_sb = work.tile([128, 2, SQ_T], bf, tag="rbsb")
                for hp in range(2):
                    rb = aux_ps.tile([128, SQ_T], f32, tag="aux")
                    pe, po_ = 64 * hp, 64 * hp + 32
                    nc.tensor.matmul(rb[0:64, :], ones_sb[pe:pe + 1, 0:64],
                                     r4[pe:pe + 1, :], start=True, stop=True,
                                     tile_position=(pe, 0))
                    nc.tensor.matmul(rb[64:128, :], ones_sb[po_:po_ + 1, 64:128],
                                     r4[po_:po_ + 1, :], start=True, stop=True,
                                     tile_position=(po_, 64))
                    nc.vector.tensor_copy(rb_sb[:, hp, :], rb[:])
                    nc.vector.tensor_mul(att_sb[:, hp, jsl], att_sb[:, hp, jsl],
                                         rb_sb[:, hp, :])
                # ---- output projection for this q-tile ----
                for t4 in range(SQ_T // 128):
                    r_ = j * (SQ_T // 128) + t4
                    tsl = bass.ds(j * SQ_T + t4 * 128, 128)
                    for o in range(2):
                        po = aux_ps.tile([128, SQ_T], f32, tag="aux")
                        for hp in range(2):
                            nc.tensor.matmul(po[:], att_sb[:, hp, tsl],
                                             wo_sb[:, hp, bass.ts(o, 512)],
                                             start=(hp == 0), stop=(hp == 1))
                        ost = work.tile([128, 512], f32, tag="ost")
                        nc.vector.tensor_copy(ost[:], po[:])
                        nc.sync.dma_start(out_v[r_, :, bass.ts(o, 512)], ost[:])

    nc.compile()
    return nc


def _get_nc(mask):
    key = hash(np.asarray(mask, dtype=bool).tobytes())
    if key not in _BUILT:
        cls, ptiles = _classify(mask)
        _BUILT[key] = (_build(cls, len(ptiles)), cls, ptiles)
    return _BUILT[key]


def _kernel_impl(q, k, v, attn_mask, Wq, Wk, Wv, Wo, trace=False):
    q = np.asarray(q, dtype=np.float32)
    k = np.asarray(k, dtype=np.float32)
    v = np.asarray(v, dtype=np.float32)
    nc, cls, ptiles = _get_nc(attn_mask)

    if ptiles:
        keep_packed = np.concatenate(ptiles, axis=0)
    else:
        keep_packed = np.zeros((SK_T, SQ_T), dtype=BF16)

    xt = {}
    for b in range(B):
        xt[("q", b)] = np.ascontiguousarray(q[b].T.astype(BF16))
        xt[("k", b)] = np.ascontiguousarray(k[b].T.astype(BF16))
        xt[("v", b)] = np.ascontiguousarray(v[b].T.astype(BF16))
    wslices = {}
    for g in range(GROUPS):
        fsl = slice(g * FPG, (g + 1) * FPG)
        wslices[("wq", g)] = np.ascontiguousarray(Wq[fsl, :].T.astype(BF16))
        wslices[("wk", g)] = np.ascontiguousarray(Wk[fsl, :].T.astype(BF16))
        wslices[("wv", g)] = np.ascontiguousarray(Wv[fsl, :].T.astype(BF16))
        wslices[("wo", g)] = np.ascontiguousarray(Wo[:, fsl].T.astype(BF16))

    in_maps = []
    for core in range(NCORES):
        b, g = core // GROUPS, core % GROUPS
        in_maps.append({
            "xqt": xt[("q", b)], "xkt": xt[("k", b)], "xvt": xt[("v", b)],
            "wqt": wslices[("wq", g)], "wkt": wslices[("wk", g)],
            "wvt": wslices[("wv", g)], "wot": wslices[("wo", g)],
            "keep": keep_packed,
        })

    res = bass_utils.run_bass_kernel_spmd(
        nc, in_maps, core_ids=list(range(NCORES)), trace=trace)

    out = np.zeros((B, S, D), dtype=np.float32)
    for core in range(NCORES):
        out[core // GROUPS] += res.results[core]["out"]
    return out, res


def kernel(q, k, v, attn_mask, Wq, Wk, Wv, Wo):
    out, _ = _kernel_impl(q, k, v, attn_mask, Wq, Wk, Wv, Wo)
    return out
